# revision 8
# baseline (speedup 1.0000x reference)
"""CrossAttention (single-head) Trainium2 kernel, 8-core data-parallel.

Full inputs in, full output out. Batch 16 is sharded 2-per-core across 8
NeuronCores. Weight fusion removes half the projections on-device:

    scores = (q@Wq + bq)(k@Wk + bk)^T / 32
           = q @ A @ k^T  (+ per-sk bias phi)  (+ per-sq const -> softmax-invariant)
        A   = (Wq @ Wk^T) / 32          (host-side, f32)
        phi = (k @ Wk @ bq) / 32        (host-side, tiny; zero when bq=0)
    out    = attn @ (v@Wv + bv) @ Wo + bo
           = (attn @ v) @ W2 + b2       (attn rows sum to 1)
        W2  = Wv @ Wo,  b2 = bv @ Wo + bo   (host-side)

So the device only computes: G = q@A, scoresT = kT^T G, exp (+phi bias),
sums (ones-matmul), U = v^T exp, out = (U^T W2) * r + b2 — 12.9 GMAC/batch
instead of 17.2. All matmuls bf16 with f32 PSUM accumulation; activations
kept in transposed [d, s] layouts so every matmul contracts over the
partition dim with no on-chip transposes of large tensors.
"""

import sys

sys.path.insert(0, "/opt/trn_rl_repo")

import numpy as np
import ml_dtypes

import concourse.bass as bass
import concourse.mybir as mybir
import concourse.tile as tile
from concourse.bass_utils import run_bass_kernel_spmd

BF16 = mybir.dt.bfloat16
F32 = mybir.dt.float32
AF = mybir.ActivationFunctionType

N_CORES = 8
B, S, D = 16, 2048, 1024
NB = B // N_CORES          # batches per core
KC = D // 128              # 8 chunks of 128 along d
ST = S // 128              # 16 tiles of 128 along s
NBLK = S // 512            # 4 blocks of 512 along s
SCALE = 1.0 / np.sqrt(np.float32(D))  # 1/32


def _split_waits(nc, limit=1):
    """Walrus in this container allows at most one sync wait per instruction:
    hoist excess waits onto NoOp carriers inserted just before."""
    n_new = 0
    for f in nc.m.functions:
        for bb in f.blocks:
            new_insts = []
            for inst in bb.instructions:
                si = inst.sync_info
                waits = list(si.on_wait) if si and si.on_wait else []
                if len(waits) > limit:
                    excess, keep = waits[:-limit], waits[-limit:]
                    for i in range(0, len(excess), limit):
                        chunk = excess[i:i + limit]
                        nop = mybir.InstNoOp(
                            name=f"{inst.name}-ws-{n_new}",
                            ins=[], outs=[],
                            sync_info=mybir.SyncInfo(on_wait=chunk, on_update=[]),
                        )
                        nop.engine = inst.engine
                        new_insts.append(nop)
                        n_new += 1
                    si.on_wait = keep
                new_insts.append(inst)
            bb.instructions[:] = new_insts
    return n_new


def _strip_dead_pe_updates(nc):
    """Drop PE sem increments nobody waits on (Tile emits one per matmul;
    only group-stop indices are ever waited). Renumber wait thresholds by
    rank among kept updates — release timing is identical, PE saves ~26ns
    per dropped serialized EVT_SEM write. Straight-line programs only."""
    pe = mybir.EngineType.PE
    insts = [i for f in nc.m.functions for bb in f.blocks for i in bb.instructions]
    upd_by_sem, wait_by_sem, bad = {}, {}, set()
    for inst in insts:
        si = inst.sync_info
        if not si:
            continue
        for u in (si.on_update or []):
            if u.sync_type != "semaphore":
                continue
            if inst.engine != pe or u.update_mode != "sem-inc" or u.update_value != 1:
                bad.add(u.id)
            upd_by_sem.setdefault(u.id, []).append((inst, u))
        for w in (si.on_wait or []):
            if w.sync_type != "semaphore":
                continue
            if w.wait_mode != "sem-ge-imm" or w.wait_reg is not None:
                bad.add(w.id)
            wait_by_sem.setdefault(w.id, []).append(w)
    n_drop = 0
    for sem_id, ups in upd_by_sem.items():
        if sem_id in bad or sem_id not in wait_by_sem or len(ups) < 16:
            continue
        waited = sorted({w.wait_value for w in wait_by_sem[sem_id]})
        if not waited or waited[-1] > len(ups) or waited[0] < 1:
            continue
        keep = set(waited)
        rank = {t: k + 1 for k, t in enumerate(waited)}
        for idx, (inst, u) in enumerate(ups, start=1):
            if idx not in keep:
                inst.sync_info.on_update = [
                    x for x in inst.sync_info.on_update if x is not u
                ]
                n_drop += 1
        for w in wait_by_sem[sem_id]:
            w.wait_value = rank[w.wait_value]
    return n_drop


def build_program(reps=1, postproc=True):
    """reps>1 wraps the whole computation in a hardware For_i loop — used
    only for timing (slope over reps isolates on-silicon exec time from
    per-call NEFF load overhead)."""
    nc = bass.Bass()

    qT_d = nc.declare_dram_parameter("qT", [NB, D, S], BF16, isOutput=False)
    kT_d = nc.declare_dram_parameter("kT", [NB, D, S], BF16, isOutput=False)
    vN_d = nc.declare_dram_parameter("vN", [NB, S, D], BF16, isOutput=False)
    A_d = nc.declare_dram_parameter("A", [D, D], BF16, isOutput=False)
    W2_d = nc.declare_dram_parameter("W2", [D, D], BF16, isOutput=False)
    phi_d = nc.declare_dram_parameter("phi", [NB, 128, ST], F32, isOutput=False)
    b2_d = nc.declare_dram_parameter("b2", [D], BF16, isOutput=False)
    out_d = nc.declare_dram_parameter("out", [NB, S, D], F32, isOutput=True)

    from contextlib import ExitStack
    with tile.TileContext(nc) as tc:
        with ExitStack() as _stk:
            _p = lambda **kw: _stk.enter_context(tc.tile_pool(**kw))
            wpool = _p(name="wts", bufs=16)
            kpool = _p(name="keyT", bufs=8)
            vpool = _p(name="value", bufs=1)
            inpool = _p(name="inp", bufs=8)
            qpool = _p(name="G", bufs=10)
            epool = _p(name="expT", bufs=2)
            upool = _p(name="UT", bufs=2)
            opool = _p(name="outb", bufs=2)
            sumpool = _p(name="sums", bufs=1)
            rpool = _p(name="rpool", bufs=2)
            phpool = _p(name="phi", bufs=2)
            cpool = _p(name="const", bufs=1)
            pspool = _p(name="ps", bufs=5, space="PSUM")
            ps1pool = _p(name="ps1", bufs=1, space="PSUM")
            psrpool = _p(name="psr", bufs=2, space="PSUM")
            # constants
            ones = cpool.tile([128, 1], BF16, tag="ones")
            nc.vector.memset(ones[:], 1.0)
            ident = cpool.tile([1, 1], F32, tag="ident")
            nc.vector.memset(ident[:], 1.0)
            b2_sb = cpool.tile([128, D], BF16, tag="b2")
            ap = b2_d[:]
            nc.sync.dma_start(
                out=b2_sb[:],
                in_=bass.AP(tensor=ap.tensor, offset=ap.offset, ap=[[0, 128]] + ap.ap),
            )

            def load_w(w_d, tag):
                tiles = []
                for i in range(KC):
                    t = wpool.tile([128, D], BF16, tag=tag, name=f"{tag}{i}")
                    nc.sync.dma_start(out=t[:], in_=w_d[i * 128:(i + 1) * 128, :])
                    tiles.append(t)
                return tiles

            # A and W2 stay resident for the whole kernel
            A_t = load_w(A_d, "wA")
            W2_t = load_w(W2_d, "w2")

            import contextlib
            loop_ctx = tc.For_i(0, reps, 1) if reps > 1 else contextlib.nullcontext()
            with loop_ctx:
              for b in range(NB):
                  # ---- raw kT tiles [d, sk] and v tiles [sk, d]: plain DMA ----
                  keyT = [kpool.tile([128, S], BF16, tag="keyT", name=f"keyT{i}") for i in range(KC)]
                  for i in range(KC):
                      nc.sync.dma_start(out=keyT[i][:], in_=kT_d[b, i * 128:(i + 1) * 128, :])
                  val = vpool.tile([128, ST, D], BF16, tag="value")
                  for t16 in range(ST):
                      nc.sync.dma_start(
                          out=val[:, t16, :],
                          in_=vN_d[b, t16 * 128:(t16 + 1) * 128, :],
                      )
                  phi_sb = phpool.tile([128, ST], F32, tag="phi")
                  nc.sync.dma_start(out=phi_sb[:], in_=phi_d[b])

                  # ---------------- per 512-wide sq block ----------------
                  for blk in range(NBLK):
                      # G block [d', 512] = A.T @ qT_blk  (A pre-scaled by 1/32)
                      qin = []
                      for i in range(KC):
                          t = inpool.tile([128, 512], BF16, tag="inp", name=f"in{i}")
                          nc.sync.dma_start(
                              out=t[:],
                              in_=qT_d[b, i * 128:(i + 1) * 128, blk * 512:(blk + 1) * 512],
                          )
                          qin.append(t)
                      qry = []
                      for do in range(KC):
                          psum = pspool.tile([128, 512], F32, tag="ps")
                          for i in range(KC):
                              nc.tensor.matmul(
                                  psum[:], A_t[i][:, do * 128:(do + 1) * 128], qin[i][:],
                                  start=(i == 0), stop=(i == KC - 1),
                              )
                          qt = qpool.tile([128, 512], BF16, tag="G", name=f"qry{do}")
                          nc.vector.tensor_copy(qt[:], psum[:])
                          qry.append(qt)

                      # scoresT -> expT, with per-sk bias phi folded into exp
                      exp_blk = epool.tile([128, ST, 512], BF16, tag="expT")
                      for t16 in range(ST):
                          psum = pspool.tile([128, 512], F32, tag="ps")
                          for i in range(KC):
                              nc.tensor.matmul(
                                  psum[:],
                                  keyT[i][:, t16 * 128:(t16 + 1) * 128],
                                  qry[i][:],
                                  start=(i == 0), stop=(i == KC - 1),
                              )
                          nc.scalar.activation(
                              exp_blk[:, t16, :], psum[:], AF.Exp,
                              bias=phi_sb[:, t16:t16 + 1],
                          )

                      # column sums over all sk (partition dim) via ones-matmul
                      sums_ps = ps1pool.tile([1, 512], F32, tag="ps1")
                      for t16 in range(ST):
                          nc.tensor.matmul(
                              sums_ps[:], ones[:], exp_blk[:, t16, :],
                              start=(t16 == 0), stop=(t16 == ST - 1),
                          )
                      sums_sb = sumpool.tile([1, 512], F32, tag="sums")
                      nc.vector.tensor_copy(sums_sb[:], sums_ps[:])

                      # r = 1/sums as per-partition scalars, via [1,128] PE
                      # transpose; emitted before UT so its PE<->DVE chain is
                      # hidden under the UT matmul stream
                      r_sb = rpool.tile([128, 4], F32, tag="r")
                      for m in range(4):
                          pr = psrpool.tile([128, 1], F32, tag="psr")
                          nc.tensor.transpose(
                              pr[:], sums_sb[0:1, m * 128:(m + 1) * 128], ident[:]
                          )
                          nc.vector.reciprocal(r_sb[:, m:m + 1], pr[:])

                      # UT block [d, 512] = v.T @ expT
                      ut = upool.tile([128, KC, 512], BF16, tag="UT")
                      for j in range(KC):
                          psum = pspool.tile([128, 512], F32, tag="ps")
                          for t16 in range(ST):
                              nc.tensor.matmul(
                                  psum[:],
                                  val[:, t16, j * 128:(j + 1) * 128],
                                  exp_blk[:, t16, :],
                                  start=(t16 == 0), stop=(t16 == ST - 1),
                              )
                          nc.vector.tensor_copy(ut[:, j, :], psum[:])

                      # final block: out[sq, d] = (UT.T @ W2) * r + b2
                      for m in range(4):
                          ob = opool.tile([128, D], F32, tag="outb")
                          for n in range(2):
                              psum = pspool.tile([128, 512], F32, tag="ps")
                              for j in range(KC):
                                  nc.tensor.matmul(
                                      psum[:],
                                      ut[:, j, m * 128:(m + 1) * 128],
                                      W2_t[j][:, n * 512:(n + 1) * 512],
                                      start=(j == 0), stop=(j == KC - 1),
                                  )
                              nc.vector.tensor_scalar_mul(
                                  ob[:, n * 512:(n + 1) * 512], psum[:], r_sb[:, m:m + 1]
                              )
                              nc.vector.tensor_add(
                                  ob[:, n * 512:(n + 1) * 512],
                                  ob[:, n * 512:(n + 1) * 512],
                                  b2_sb[:, n * 512:(n + 1) * 512],
                              )
                          sq = blk * 512 + m * 128
                          nc.sync.dma_start(out=out_d[b, sq:sq + 128, :], in_=ob[:])

    if postproc:
        if reps == 1:
            _strip_dead_pe_updates(nc)
        _split_waits(nc)
    return nc


_PROGRAM = None


def _get_program():
    global _PROGRAM
    if _PROGRAM is None:
        _PROGRAM = build_program()
    return _PROGRAM


def prepare_in_maps(q, k, v, Wq, bq, Wk, bk, Wv, bv, Wo, bo):
    bf = ml_dtypes.bfloat16
    f32 = np.float32

    def t_bf16(x):  # [B,S,D] f32 -> [B,D,S] bf16 contiguous
        return np.ascontiguousarray(
            np.asarray(x, f32).astype(bf).transpose(0, 2, 1)
        )

    qT = t_bf16(q)
    kT = t_bf16(k)
    vN = np.ascontiguousarray(np.asarray(v, f32).astype(bf))
    Wq_f = np.asarray(Wq, f32)
    Wk_f = np.asarray(Wk, f32)
    Wv_f = np.asarray(Wv, f32)
    Wo_f = np.asarray(Wo, f32)
    bq_f = np.asarray(bq, f32)
    bo_f = np.asarray(bo, f32)
    bv_f = np.asarray(bv, f32)
    A = ((Wq_f @ Wk_f.T) * np.float32(SCALE)).astype(bf)
    W2 = (Wv_f @ Wo_f).astype(bf)
    b2 = (bv_f @ Wo_f + bo_f).astype(bf)
    # phi[b, sk] = (k[b] @ Wk @ bq) / 32, laid out [128, ST] per batch
    phi_vec = np.einsum(
        "bsd,d->bs", np.asarray(k, f32), Wk_f @ bq_f
    ) * np.float32(SCALE)
    phi = np.ascontiguousarray(
        phi_vec.reshape(B, ST, 128).transpose(0, 2, 1)
    ).astype(f32)

    in_maps = []
    for c in range(N_CORES):
        sl = slice(c * NB, (c + 1) * NB)
        in_maps.append({
            "qT": qT[sl], "kT": kT[sl], "vN": vN[sl],
            "A": A, "W2": W2, "phi": phi[sl], "b2": b2,
        })
    return in_maps


def kernel(q, k, v, Wq, bq, Wk, bk, Wv, bv, Wo, bo):
    nc = _get_program()
    in_maps = prepare_in_maps(q, k, v, Wq, bq, Wk, bk, Wv, bv, Wo, bo)
    res = run_bass_kernel_spmd(nc, in_maps, core_ids=list(range(N_CORES)))
    out = np.concatenate([res.results[c]["out"] for c in range(N_CORES)], axis=0)
    return out.astype(np.float32)
